# revision 1
# baseline (speedup 1.0000x reference)
"""Trainium2 Bass kernel for nn_MCPBRNN_constant_OutLoss.

Math: the reference runs a linear recurrence c[k] = f*c[k-1] + u[k] over the
flattened (B*T,) channel-0 input and samples it at k = 4b+2.  Sampling at row
rate gives a per-row recurrence c0[b] = a*c0[b-1] + v[b] with a = f^4 and
v[b] = f^2*x[b,0,0] + f*x[b,1,0] + x[b,2,0] + f^3*x[b-1,3,0].

Since f = softmax(w)_f <= e/(e+2) ~ 0.576, a <= 0.111 and a^32 ~ 1e-30, the
scan has an effective history of < 32 rows in float32.  Each of the 8 cores
therefore processes a contiguous slice of rows fully independently, seeding
its local scan with a 32-row halo (exact to f32 precision) - no cross-core
carry exchange is needed.  Inside a core, rows are laid out 1024-per-partition
and the scan runs along the free dimension with the native hardware
tensor_tensor_scan instruction, chained across chunks via its `initial` input;
the 32-row halo seeds each partition.

Outputs: c_n = c0, h_n = oo*c0, l_n = ol*c0, h_nout = [h_n, obs_std],
constant gate planes, zeros, and obs_std = std(y_obs[1000:500000], ddof=1)
(computed on device via sum/sumsq reduction + cross-partition DMA transpose).
"""
import os
import numpy as np

import concourse.bass as bass
import concourse.tile as tile
from concourse import mybir
from concourse.bass_utils import run_bass_kernel_spmd

f32 = mybir.dt.float32

B = 1048576
NCORES = 8
BC = B // NCORES          # 131072 rows per core
P = 128
PPB = BC // P             # 1024 rows per partition
CHUNK = 256
NCHUNK = PPB // CHUNK     # 4
HALO = 32                 # scan history halo (a^32 ~ 1e-30)
XPAD = HALO + 1           # x rows of padding ahead of each core slice

SPIN, TRAIN = 1000, 500000
YN = TRAIN - SPIN         # 499000
YF = 3899                 # 128*3899 = 499072 >= YN (zero padded)

LAST_RESULTS = [None]     # BassKernelResults stash for test harness


def _split_multi_waits(nc):
    """This walrus build supports only ONE sync wait per instruction; hoist
    extra waits onto single-wait NoOps spliced before the instruction."""
    ctr = 0
    for f in nc.m.functions:
        for blk in f.blocks:
            out = []
            changed = False
            for ins in blk.instructions:
                si = ins.sync_info
                if si is not None and si.on_wait and len(si.on_wait) > 1:
                    w = list(si.on_wait)
                    for extra in w[:-1]:
                        ctr += 1
                        nop = mybir.InstNoOp(name=f"WSPLIT-{ctr}", ins=[], outs=[])
                        nop.engine = ins.engine
                        nop.sync_info = mybir.SyncInfo(on_wait=[extra], on_update=[])
                        out.append(nop)
                    ins.sync_info = mybir.SyncInfo(
                        on_wait=[w[-1]], on_update=list(si.on_update or []))
                    changed = True
                out.append(ins)
            if changed:
                blk.instructions = out


def _build_nc(oo, ol, f):
    """Build the single-core SPMD program (same NEFF on all 8 cores)."""
    oo = float(oo)
    ol = float(ol)
    f = float(f)
    f2, f3, a = f * f, f * f * f, f * f * f * f

    nc = bass.Bass()
    xc = nc.dram_tensor("xc", (BC + XPAD, 8), f32, kind="ExternalInput")
    yp = nc.dram_tensor("yp", (P, YF), f32, kind="ExternalInput")
    o_h = nc.dram_tensor("h_n", (BC, 1), f32, kind="ExternalOutput")
    o_c = nc.dram_tensor("c_n", (BC, 1), f32, kind="ExternalOutput")
    o_l = nc.dram_tensor("l_n", (BC, 1), f32, kind="ExternalOutput")
    o_bp = nc.dram_tensor("bp_n", (BC, 1), f32, kind="ExternalOutput")
    o_gib = nc.dram_tensor("gib", (BC, 1), f32, kind="ExternalOutput")
    o_goo = nc.dram_tensor("goo", (BC, 1), f32, kind="ExternalOutput")
    o_gol = nc.dram_tensor("gol", (BC, 1), f32, kind="ExternalOutput")
    o_gf = nc.dram_tensor("gf", (BC, 1), f32, kind="ExternalOutput")
    o_std = nc.dram_tensor("ostd", (BC, 1), f32, kind="ExternalOutput")
    o_hn = nc.dram_tensor("h_nout", (BC, 2), f32, kind="ExternalOutput")

    def col(dram, k):  # (BC,1) output -> (128, PPB) view, chunk k columns
        v = dram[:, :].rearrange("(p n) o -> p (n o)", p=P)
        return v[:, k * CHUNK:(k + 1) * CHUNK]

    with tile.TileContext(nc) as tc:
        with (tc.tile_pool(name="xin", bufs=3) as xin,
              tc.tile_pool(name="vbuf", bufs=3) as vbuf,
              tc.tile_pool(name="outb", bufs=3) as outb,
              tc.tile_pool(name="hnb", bufs=3) as hnb,
              tc.tile_pool(name="stage", bufs=1) as stage,
              tc.tile_pool(name="consts", bufs=1) as consts,
              tc.tile_pool(name="ybuf", bufs=1) as ybuf,
              tc.tile_pool(name="dram", bufs=1, space="DRAM") as dpool):

            # ---- constant tiles ----
            a_t = consts.tile([P, CHUNK + HALO], f32)
            nc.vector.memset(a_t[:, :], a)
            zero_t = consts.tile([P, PPB], f32)
            nc.gpsimd.memset(zero_t[:, :], 0.0)
            goo_t = consts.tile([P, PPB], f32)
            nc.gpsimd.memset(goo_t[:, :], oo)
            gol_t = consts.tile([P, PPB], f32)
            nc.gpsimd.memset(gol_t[:, :], ol)
            gf_t = consts.tile([P, PPB], f32)
            nc.gpsimd.memset(gf_t[:, :], f)

            # ---- obs_std pipeline ----
            y_t = ybuf.tile([P, YF], f32)
            nc.sync.dma_start(out=y_t[:, :], in_=yp[:, :])
            red = ybuf.tile([P, 2], f32)
            nc.vector.reduce_sum(out=red[:, 0:1], in_=y_t[:, :],
                                 axis=mybir.AxisListType.X)
            sq_t = ybuf.tile([P, YF], f32)
            nc.scalar.activation(out=sq_t[:, :], in_=y_t[:, :],
                                 func=mybir.ActivationFunctionType.Square,
                                 accum_out=red[:, 1:2])
            scr = dpool.tile([P, 2], f32)
            nc.sync.dma_start(out=scr[:, :], in_=red[:, :])
            t2 = ybuf.tile([1, 2 * P], f32)
            nc.sync.dma_start(out=t2[0:1, 0:P], in_=scr[:, 0:1].rearrange("a b -> b a"))
            nc.sync.dma_start(out=t2[0:1, P:2 * P], in_=scr[:, 1:2].rearrange("a b -> b a"))
            stot = ybuf.tile([1, 2], f32)
            nc.vector.reduce_sum(out=stot[0:1, 0:1], in_=t2[0:1, 0:P],
                                 axis=mybir.AxisListType.X)
            nc.vector.reduce_sum(out=stot[0:1, 1:2], in_=t2[0:1, P:2 * P],
                                 axis=mybir.AxisListType.X)
            var = ybuf.tile([1, 1], f32)
            # var = SS/(n-1) - S*S/(n*(n-1))
            nc.vector.tensor_scalar(
                out=var[0:1, :], in0=stot[0:1, 0:1], scalar1=stot[0:1, 0:1],
                scalar2=-1.0 / (float(YN) * (YN - 1)),
                op0=mybir.AluOpType.mult, op1=mybir.AluOpType.mult)
            nc.vector.scalar_tensor_tensor(
                out=var[0:1, :], in0=stot[0:1, 1:2], scalar=1.0 / (YN - 1),
                in1=var[0:1, :], op0=mybir.AluOpType.mult, op1=mybir.AluOpType.add)
            nc.scalar.sqrt(out=var[0:1, :], in_=var[0:1, :])
            scr2 = dpool.tile([1, 1], f32)
            nc.sync.dma_start(out=scr2[:, :], in_=var[0:1, :])
            std_col = consts.tile([P, 1], f32)
            nc.sync.dma_start(out=std_col[:, :], in_=scr2[:, :].to_broadcast([P, 1]))
            ostd_t = consts.tile([P, PPB], f32)
            nc.scalar.add(out=ostd_t[:, :], in_=zero_t[:, :], add=std_col[:, 0:1])

            # ---- main scan over 4 chunks ----
            c_stage = stage.tile([P, HALO + PPB], f32)
            for k in range(NCHUNK):
                halo_k = HALO if k == 0 else 0
                R = CHUNK + halo_k
                XR = R + 1
                row0 = k * CHUNK + HALO - halo_k
                xt = xin.tile([P, CHUNK + HALO + 1, 8], f32, tag="xt")
                src = bass.AP(tensor=xc[:, :].tensor, offset=row0 * 8,
                              ap=[[PPB * 8, P], [8, XR], [1, 8]])
                nc.sync.dma_start(out=xt[:, :XR, :], in_=src)

                vt = vbuf.tile([P, CHUNK + HALO], f32, tag="vt")
                nc.vector.scalar_tensor_tensor(
                    out=vt[:, :R], in0=xt[:, 1:XR, 0], scalar=f2, in1=xt[:, 1:XR, 4],
                    op0=mybir.AluOpType.mult, op1=mybir.AluOpType.add)
                nc.vector.scalar_tensor_tensor(
                    out=vt[:, :R], in0=xt[:, 1:XR, 2], scalar=f, in1=vt[:, :R],
                    op0=mybir.AluOpType.mult, op1=mybir.AluOpType.add)
                nc.vector.scalar_tensor_tensor(
                    out=vt[:, :R], in0=xt[:, 0:R, 6], scalar=f3, in1=vt[:, :R],
                    op0=mybir.AluOpType.mult, op1=mybir.AluOpType.add)

                dst = c_stage[:, k * CHUNK + HALO - halo_k: HALO + (k + 1) * CHUNK]
                init = (0.0 if k == 0 else
                        c_stage[:, HALO + k * CHUNK - 1: HALO + k * CHUNK])
                nc.vector.tensor_tensor_scan(
                    out=dst, data0=a_t[:, :R], data1=vt[:, :R], initial=init,
                    op0=mybir.AluOpType.mult, op1=mybir.AluOpType.add)

                c_real = c_stage[:, HALO + k * CHUNK: HALO + (k + 1) * CHUNK]
                h_t = outb.tile([P, CHUNK], f32, tag="ht")
                nc.scalar.mul(out=h_t[:, :], in_=c_real, mul=oo)
                l_t = outb.tile([P, CHUNK], f32, tag="lt")
                nc.scalar.mul(out=l_t[:, :], in_=c_real, mul=ol)
                hn_t = hnb.tile([P, CHUNK, 2], f32, tag="hnt")
                nc.scalar.mul(out=hn_t[:, :, 0], in_=c_real, mul=oo)
                nc.scalar.add(out=hn_t[:, :, 1], in_=zero_t[:, :CHUNK],
                              add=std_col[:, 0:1])

                nc.sync.dma_start(out=col(o_c, k), in_=c_real)
                nc.sync.dma_start(out=col(o_h, k), in_=h_t[:, :])
                nc.sync.dma_start(out=col(o_l, k), in_=l_t[:, :])
                hn_view = o_hn[:, :].rearrange("(p n) e -> p n e", p=P)
                nc.sync.dma_start(out=hn_view[:, k * CHUNK:(k + 1) * CHUNK, :],
                                  in_=hn_t[:, :, :])

            # ---- constant outputs ----
            full = lambda dram: dram[:, :].rearrange("(p n) o -> p (n o)", p=P)
            nc.sync.dma_start(out=full(o_bp), in_=zero_t[:, :])
            nc.sync.dma_start(out=full(o_gib), in_=zero_t[:, :])
            nc.sync.dma_start(out=full(o_goo), in_=goo_t[:, :])
            nc.sync.dma_start(out=full(o_gol), in_=gol_t[:, :])
            nc.sync.dma_start(out=full(o_gf), in_=gf_t[:, :])
            nc.sync.dma_start(out=full(o_std), in_=ostd_t[:, :])

    _split_multi_waits(nc)
    return nc


def kernel(x, y_obs, weight_r_yom, weight_r_ylm, weight_r_yfm, epoch, time_lag):
    del epoch
    assert int(time_lag) == 0
    x = np.asarray(x, dtype=np.float32)
    y_obs = np.asarray(y_obs, dtype=np.float32)
    wom = np.asarray(weight_r_yom, dtype=np.float32).reshape(())
    wlm = np.asarray(weight_r_ylm, dtype=np.float32).reshape(())
    wfm = np.asarray(weight_r_yfm, dtype=np.float32).reshape(())

    # constant softmax gates (match reference's f32 arithmetic)
    eo, el, ef = np.exp(wom), np.exp(wlm), np.exp(wfm)
    denom = eo + el + ef
    oo = np.float32(eo / denom)
    ol = np.float32(el / denom)
    f = np.float32(1.0) - oo - ol

    # host-side sharding: pad 33 zero rows ahead so every partition of every
    # core can read its 32-row scan halo (+1 row for the x[b-1,3,0] shift)
    xf = x.reshape(B, 8)
    x_pad = np.zeros((B + XPAD, 8), dtype=np.float32)
    x_pad[XPAD:] = xf
    y_pad = np.zeros((P * YF,), dtype=np.float32)
    y_pad[:YN] = y_obs[SPIN:TRAIN, 0]
    y_pad = y_pad.reshape(P, YF)

    in_maps = [{"xc": np.ascontiguousarray(x_pad[m * BC: (m + 1) * BC + XPAD]),
                "yp": y_pad} for m in range(NCORES)]

    nc = _build_nc(oo, ol, f)
    trace = bool(int(os.environ.get("KERNEL_TRACE", "0")))
    res = run_bass_kernel_spmd(nc, in_maps, core_ids=list(range(NCORES)),
                               trace=trace)
    LAST_RESULTS[0] = res

    cat = lambda name: np.concatenate([res.results[m][name]
                                       for m in range(NCORES)], axis=0)
    h_n = cat("h_n")
    c_n = cat("c_n")
    l_n = cat("l_n")
    bp_n = cat("bp_n")
    gib = cat("gib")
    goo = cat("goo")
    gol = cat("gol")
    gf = cat("gf")
    h_nout = cat("h_nout")
    ostd = cat("ostd")
    return (h_n, c_n, l_n, bp_n, gib, goo, gol, gf, h_nout, ostd)
